# revision 13
# baseline (speedup 1.0000x reference)
import functools

import jax
import jax.numpy as jnp
import numpy as np

KH, KW = 3, 3
STRIDE, PAD, DIL = 1, 1, 1


def _conv(x, w):
    return jax.lax.conv_general_dilated(
        x, w, (STRIDE, STRIDE), [(PAD, PAD), (PAD, PAD)],
        rhs_dilation=(DIL, DIL), dimension_numbers=('NCHW', 'OIHW', 'NCHW'))


def _deform_sample(x, py, px):
    # x: [B,C,H,W]; py,px: [B,K,Ho,Wo] absolute fractional sample coords.
    B, C, H, W = x.shape
    y0 = jnp.floor(py)
    x0 = jnp.floor(px)
    wy1 = py - y0
    wx1 = px - x0
    xf = x.reshape(B, C, H * W)
    out = jnp.zeros((B, C) + py.shape[1:], dtype=x.dtype)
    corners = [
        (y0,     x0,     (1 - wy1) * (1 - wx1)),
        (y0,     x0 + 1, (1 - wy1) * wx1),
        (y0 + 1, x0,     wy1 * (1 - wx1)),
        (y0 + 1, x0 + 1, wy1 * wx1),
    ]
    for yc, xc, wgt in corners:
        valid = (yc >= 0) & (yc < H) & (xc >= 0) & (xc < W)
        yi = jnp.clip(yc, 0, H - 1).astype(jnp.int32)
        xi = jnp.clip(xc, 0, W - 1).astype(jnp.int32)
        idx = (yi * W + xi).reshape(B, -1)
        g = jax.vmap(lambda im, ix: im[:, ix])(xf, idx)
        out = out + g.reshape(B, C, *py.shape[1:]) * (wgt * valid)[:, None]
    return out  # [B, C, K, Ho, Wo]


def _forward(x_real, x_imag, w_offset, w_mag, w_real, w_imag, b_real, b_imag):
    B, C, H, W = x_real.shape
    K = KH * KW
    Cout = w_real.shape[0]
    off = _conv(jnp.concatenate([x_real, x_imag], axis=1), w_offset)
    mag = jax.nn.sigmoid(_conv(jnp.sqrt(x_real * x_real + x_imag * x_imag), w_mag))
    Ho, Wo = off.shape[2], off.shape[3]
    off = off.reshape(B, K, 2, Ho, Wo)
    dy, dx = off[:, :, 0], off[:, :, 1]
    grid_y = jnp.repeat(jnp.arange(KH) * DIL, KW).astype(jnp.float32)
    grid_x = jnp.tile(jnp.arange(KW) * DIL, KH).astype(jnp.float32)
    base_y = (jnp.arange(Ho) * STRIDE - PAD).astype(jnp.float32)
    base_x = (jnp.arange(Wo) * STRIDE - PAD).astype(jnp.float32)
    py = base_y[None, None, :, None] + grid_y[None, :, None, None] + dy
    px = base_x[None, None, None, :] + grid_x[None, :, None, None] + dx
    xc = jnp.concatenate([x_real, x_imag], axis=1)
    samp = _deform_sample(xc, py, px) * mag[:, None]
    sr, si = samp[:, :C], samp[:, C:]
    Wr = w_real.reshape(Cout, C, K)
    Wi = w_imag.reshape(Cout, C, K)
    cv = lambda Wm, s: jnp.einsum('ock,bckhw->bohw', Wm, s)
    real = cv(Wr, sr) - cv(Wi, si) + (b_real - b_imag)[None, :, None, None]
    imag = cv(Wi, sr) - cv(Wr, si) + (b_imag - b_real)[None, :, None, None]
    return jnp.stack([real, imag], axis=-1)


@functools.cache
def _compiled():
    # Data-parallel over batch across the 8 NeuronCores; weights broadcast.
    return jax.pmap(_forward, in_axes=(0, 0, None, None, None, None, None, None))


def kernel(x_real, x_imag, w_offset, w_mag, w_real, w_imag, b_real, b_imag):
    B = x_real.shape[0]
    n_dev = min(len(jax.devices()), B)
    fn = _compiled()
    # [B,...] -> [n_dev, B/n_dev, ...]
    xr = x_real.reshape(n_dev, B // n_dev, *x_real.shape[1:])
    xi = x_imag.reshape(n_dev, B // n_dev, *x_imag.shape[1:])
    out = fn(xr, xi, w_offset, w_mag, w_real, w_imag, b_real, b_imag)
    out = np.asarray(out)
    return out.reshape(B, *out.shape[2:]).astype(np.float32)
